# revision 1
# baseline (speedup 1.0000x reference)
"""Trainium2 Bass kernel for nn_PhysicsPriorExtractor.

Reference computation per batch element (B=32768, T=64, K*D=30 features):
  spatial = pose reshaped [T, 30]                          -> out channels 0..29
  vel     = bidirectional-EMA(diff(spatial)/clip(dt))      -> out channels 30..59
  mzeni   = dot(ankle_l - ankle_r, forward_dir) + 1e-6     -> out channel 60

Strategy (pure data-parallel, batch sharded 8 ways):
  * batch on SBUF partitions, CPT=2 batch rows per partition per tile
  * EMA scans via the DVE TensorTensorScan instruction, all 30 features
    (x CPT batches) in ONE scan: feature-major [F, T] free layout with the
    recurrence reset at segment starts via data0=0 there (state = d0*state + d1)
  * backward scan runs on a time-reversed copy (ACT engine negative-stride
    copy); the combine pass reads it back reversed
  * everything assembled into a [128, CPT*64*61] output tile, one big
    contiguous DMA per tile -> memory-bound roofline
"""

import sys

if "/opt/trn_rl_repo" not in sys.path:
    sys.path.insert(0, "/opt/trn_rl_repo")

import numpy as np

B, T, F = 32768, 64, 30
OC = 2 * F + 1  # 61 output channels
N_CORES = 8
BC = B // N_CORES  # 4096 batch rows per core
CPT = 2  # batch rows per partition per tile
ROWS = 128 * CPT  # batch rows per tile
ALPHA = 0.7
Q = 1.0 - ALPHA
MAX_DT = 0.1

FT = F * T  # 1920, per-batch free width (scan layout)
W = CPT * FT  # 3840
OW = CPT * T * OC  # 7808


class _Null:
    def __getattr__(self, _name):
        return lambda *a, **k: None


def build_nc(bc=BC, repeat=1, loop_repeat=1, parts=None):
    """Build the single-core Bass program processing a [bc, T, F] shard.

    repeat>1 unrolls the whole body that many times; loop_repeat>1 wraps it
    in a hardware For_i loop (timing-only variants; extra passes recompute
    the same output). parts (timing-only) selects instruction subsets from
    {"dma", "dve", "pool", "act", "mz"}; None = all (the real kernel)."""
    on = (
        {"dma", "dve", "pool", "act", "mz"}
        if parts is None
        else set(parts)
    )
    import concourse.bacc as bacc
    import concourse.mybir as mybir
    from concourse import tile
    from concourse.ap import AP

    f32 = mybir.dt.float32
    Alu = mybir.AluOpType
    n_tiles = bc // ROWS
    assert n_tiles * ROWS == bc

    nc = bacc.Bacc(
        "TRN2", target_bir_lowering=False, debug=False, num_devices=N_CORES
    )
    pose = nc.dram_tensor("pose", [bc, FT], f32, kind="ExternalInput")
    dts = nc.dram_tensor("dt", [bc, T], f32, kind="ExternalInput")
    outd = nc.dram_tensor("out", [bc, T * OC], f32, kind="ExternalOutput")

    def mk(t, off, pairs):
        # custom free-dim access pattern on a 2D [128, N] tile
        return AP(t.tensor, t.offset + off, [list(t.ap[0])] + [list(p) for p in pairs])

    with tile.TileContext(nc) as tc:
        with (
            tc.tile_pool(name="cpool", bufs=1) as cpool,
            tc.tile_pool(name="pin", bufs=2) as pin,
            tc.tile_pool(name="pmid", bufs=1) as pmid,
            tc.tile_pool(name="pout", bufs=2) as pout,
            tc.tile_pool(name="psmall", bufs=2) as psmall,
        ):
            # scan decay tile: Q everywhere, 0 at each segment start so the
            # recurrence resets per (batch-chunk, feature) segment
            qt = cpool.tile([128, W], f32)
            nc.vector.memset(qt[:, :], Q)
            nc.vector.memset(mk(qt, 0, [[FT, CPT], [T, F]]), 0.0)

            def body():
                for i in range(n_tiles * repeat):
                    _tile_body(i)

            def _tile_body(i):
                r0 = (i % n_tiles) * ROWS
                xin = pin.tile([128, W], f32)
                dtt = psmall.tile([128, CPT * T], f32)
                rr = psmall.tile([128, CPT * T], f32)
                av = pmid.tile([128, W], f32)
                avr = pmid.tile([128, W], f32)
                yf = pmid.tile([128, W], f32)
                outt = pout.tile([128, OW], f32)

                # timing-variant-only dummy writers so no tile is
                # read-but-never-written (parts=None emits nothing here)
                if parts is not None:
                    if "dma" not in on:
                        nc.vector.memset(mk(xin, 0, [[1, 1]]), 0.0)
                        nc.vector.memset(mk(dtt, 0, [[1, 1]]), 0.0)
                    if "dve" not in on and "act" in on:
                        nc.vector.memset(mk(av, 0, [[1, 1]]), 0.0)
                    if "dma" in on and not ({"dve", "pool", "mz"} & on):
                        nc.vector.memset(mk(outt, 0, [[1, 1]]), 0.0)

                # ---- loads (ACT-issued HWDGE ring, separate from stores) ----
                if "dma" in on:
                    pose_sl = pose[r0 : r0 + ROWS, :].rearrange(
                        "(c p) f -> p c f", p=128
                    )
                    nc.scalar.dma_start(
                        out=mk(xin, 0, [[FT, CPT], [1, FT]]), in_=pose_sl
                    )
                    dt_sl = dts[r0 : r0 + ROWS, :].rearrange(
                        "(c p) t -> p c t", p=128
                    )
                    nc.scalar.dma_start(out=mk(dtt, 0, [[T, CPT], [1, T]]), in_=dt_sl)

                # ---- rr = 1 / clip(dt, 1e-6, MAX_DT) ----
                if "dve" not in on:
                    dve = _Null()
                else:
                    dve = nc.vector
                dve.tensor_scalar(
                    out=rr[:, :], in0=dtt[:, :], scalar1=MAX_DT, scalar2=1e-6,
                    op0=Alu.min, op1=Alu.max,
                )
                dve.reciprocal(rr[:, :], rr[:, :])

                # ---- av[c, f, t] = (x[c, t, f] - x[c, t-1, f]) * (0.5*alpha/dt_t)
                # x is t-major in xin; av is written f-major (T contiguous)
                av_b = mk(av, 1, [[FT, CPT], [T, F], [1, T - 1]])
                dve.tensor_tensor(
                    out=av_b,
                    in0=mk(xin, F, [[FT, CPT], [1, F], [F, T - 1]]),
                    in1=mk(xin, 0, [[FT, CPT], [1, F], [F, T - 1]]),
                    op=Alu.subtract,
                )
                # TensorScalarPtr ops are limited to 3D APs by the BIR
                # verifier -> one scale op per batch chunk
                for c in range(CPT):
                    avc = mk(av, c * FT + 1, [[T, F], [1, T - 1]])
                    dve.scalar_tensor_tensor(
                        out=avc, in0=avc, scalar=0.5 * ALPHA,
                        in1=mk(rr, c * T + 1, [[0, F], [1, T - 1]]),
                        op0=Alu.mult, op1=Alu.mult,
                    )
                # vel_0 = 0
                dve.memset(mk(av, 0, [[FT, CPT], [T, F]]), 0.0)

                # ---- time-reversed copy for the backward scan (ACT) ----
                if "act" in on:
                    nc.scalar.copy(
                        out=mk(avr, 0, [[FT, CPT], [T, F], [1, T]]),
                        in_=mk(av, T - 1, [[FT, CPT], [T, F], [-1, T]]),
                    )
                # backward initial condition: z_{T-1} = v_{T-1} (not alpha*v):
                # patch segment starts from 0.5*alpha*v to 0.5*v
                seg0 = mk(avr, 0, [[FT, CPT], [T, F]])
                dve.tensor_scalar_mul(out=seg0, in0=seg0, scalar1=1.0 / ALPHA)

                # ---- the two EMA scans, state = q*state + d1 ----
                # fwd writes a separate tile so the ACT reverse-copy (reads
                # av) overlaps instead of serializing on a WAR hazard
                dve.tensor_tensor_scan(
                    out=yf[:, :], data0=qt[:, :], data1=av[:, :], initial=0.0,
                    op0=Alu.mult, op1=Alu.add,
                )
                dve.tensor_tensor_scan(
                    out=avr[:, :], data0=qt[:, :], data1=avr[:, :], initial=0.0,
                    op0=Alu.mult, op1=Alu.add,
                )

                # ---- combine: out[c, t, 30+f] = yf[c, f, t] + yb[c, f, T-1-t]
                dve.tensor_tensor(
                    out=mk(outt, F, [[T * OC, CPT], [OC, T], [1, F]]),
                    in0=mk(yf, 0, [[FT, CPT], [1, T], [T, F]]),
                    in1=mk(avr, T - 1, [[FT, CPT], [-1, T], [T, F]]),
                    op=Alu.add,
                )

                # ---- spatial passthrough: out[c, t, f] = x[c, t, f] (ACT;
                # GpSimd measured ~+12us/tile of port contention with DVE) ----
                if "pool" in on:
                    nc.scalar.copy(
                        out=mk(outt, 0, [[T * OC, CPT], [OC, T], [1, F]]),
                        in_=mk(xin, 0, [[FT, CPT], [F, T], [1, F]]),
                    )

                # ---- M-Zeni channel ----
                # forward dir from pelvis displacement (joint 0, feats 0..2):
                # fdir = pd / (||pd|| + 63e-6), pd = p[T-1] - p[0]
                if "mz" in on:
                    pd = psmall.tile([128, CPT * 3], f32)
                    sq = psmall.tile([128, CPT * 3], f32)
                    nsq = psmall.tile([128, CPT], f32)
                    inv = psmall.tile([128, CPT], f32)
                    fd = psmall.tile([128, CPT * 3], f32)
                    ad = psmall.tile([128, CPT * T * 3], f32)
                    mzt = psmall.tile([128, CPT * T], f32)

                    pd3 = mk(pd, 0, [[3, CPT], [1, 3]])
                    nc.vector.tensor_tensor(
                        out=pd3,
                        in0=mk(xin, (T - 1) * F, [[FT, CPT], [1, 3]]),
                        in1=mk(xin, 0, [[FT, CPT], [1, 3]]),
                        op=Alu.subtract,
                    )
                    nc.vector.tensor_tensor(out=mk(sq, 0, [[3, CPT], [1, 3]]),
                                             in0=pd3, in1=pd3, op=Alu.mult)
                    nc.vector.tensor_reduce(
                        out=mk(nsq, 0, [[1, CPT]]),
                        in_=mk(sq, 0, [[3, CPT], [1, 3]]),
                        axis=mybir.AxisListType.X, op=Alu.add,
                    )
                    ns2 = mk(nsq, 0, [[1, CPT]])
                    nc.scalar.sqrt(out=ns2, in_=ns2)
                    nc.vector.tensor_scalar_add(out=ns2, in0=ns2, scalar1=(T - 1) * 1e-6)
                    nc.vector.reciprocal(mk(inv, 0, [[1, CPT]]), ns2)
                    nc.vector.tensor_tensor(
                        out=mk(fd, 0, [[3, CPT], [1, 3]]), in0=pd3,
                        in1=mk(inv, 0, [[1, CPT], [0, 3]]), op=Alu.mult,
                    )
                    # ankle_l (joint 3, feats 9..11) - ankle_r (joint 6, feats 18..20)
                    ad3 = mk(ad, 0, [[T * 3, CPT], [3, T], [1, 3]])
                    nc.vector.tensor_tensor(
                        out=ad3,
                        in0=mk(xin, 9, [[FT, CPT], [F, T], [1, 3]]),
                        in1=mk(xin, 18, [[FT, CPT], [F, T], [1, 3]]),
                        op=Alu.subtract,
                    )
                    nc.vector.tensor_tensor(
                        out=ad3, in0=ad3,
                        in1=mk(fd, 0, [[3, CPT], [0, T], [1, 3]]), op=Alu.mult,
                    )
                    nc.vector.tensor_reduce(
                        out=mk(mzt, 0, [[T, CPT], [1, T]]),
                        in_=ad3, axis=mybir.AxisListType.X, op=Alu.add,
                    )
                    nc.vector.tensor_scalar_add(
                        out=mk(outt, 2 * F, [[T * OC, CPT], [OC, T]]),
                        in0=mk(mzt, 0, [[T, CPT], [1, T]]), scalar1=1e-6,
                    )


                # ---- store (SP-issued HWDGE ring) ----
                if "dma" in on:
                    out_sl = outd[r0 : r0 + ROWS, :].rearrange(
                        "(c p) f -> p c f", p=128
                    )
                    nc.sync.dma_start(
                        out=out_sl, in_=mk(outt, 0, [[T * OC, CPT], [1, T * OC]])
                    )

            if loop_repeat > 1:
                with tc.For_i(0, loop_repeat, 1):
                    body()
            else:
                body()

    nc.compile()
    return nc


_CACHE = {}


def _get_nc():
    if "nc" not in _CACHE:
        _CACHE["nc"] = build_nc(BC)
    return _CACHE["nc"]


def kernel(pose_seq: np.ndarray, dt_seq: np.ndarray) -> np.ndarray:
    from concourse.bass_utils import run_bass_kernel_spmd

    nc = _get_nc()
    pose = np.ascontiguousarray(
        pose_seq.reshape(B, FT), dtype=np.float32
    )
    dt = np.ascontiguousarray(dt_seq.reshape(B, T), dtype=np.float32)

    in_maps = [
        {
            "pose": pose[c * BC : (c + 1) * BC],
            "dt": dt[c * BC : (c + 1) * BC],
        }
        for c in range(N_CORES)
    ]
    res = run_bass_kernel_spmd(nc, in_maps, list(range(N_CORES)))
    out = np.concatenate([r["out"] for r in res.results], axis=0)
    return out.reshape(B, T, OC)



# revision 2
# speedup vs baseline: 1.0099x; 1.0099x over previous
"""Trainium2 Bass kernel for nn_PhysicsPriorExtractor — bf16 rework.

Per batch element (B=32768, T=64, K*D=30 features):
  spatial = pose reshaped [T, 30]                          -> out channels 0..29
  vel     = bidirectional-EMA(diff(spatial)/clip(dt))      -> out channels 30..59
  mzeni   = dot(ankle_l - ankle_r, forward_dir) + 1e-6     -> out channel 60

Measured op rates (this box): DVE scan 2.37 cyc/elem any dtype; DVE TT
bf16 stride-1 2x (0.53 cyc/elem); strided TT ~2 cyc/elem; GPSIMD TT
7.2-13.2 us per 3840-elem op, ~40-60% hideable behind DVE.

Engine split per tile (CPT=2 batch rows/partition, 16 tiles/core):
  DVE : rr chain (clip/recip_approx/cast), the 2x stride-1 scale,
        fwd scan, t=63 patch, bwd scan (negative-stride APs — no reverse
        copy, output lands t-ascending), combine(+transpose) into outt,
        small mz ops
  GP  : diff (fused t->f transpose + f32->bf16 cast)
  ACT : spatial passthrough copy, mz sqrt, mz square+accum
  DMA : HWDGE loads (ACT ring), HWDGE store (SP ring)
"""

import sys

if "/opt/trn_rl_repo" not in sys.path:
    sys.path.insert(0, "/opt/trn_rl_repo")

import numpy as np

B, T, F = 32768, 64, 30
OC = 2 * F + 1
N_CORES = 8
BC = B // N_CORES
CPT = 2
ROWS = 128 * CPT
ALPHA = 0.7
Q = 1.0 - ALPHA
MAX_DT = 0.1

FT = F * T  # 1920
W = CPT * FT  # 3840
OW = CPT * T * OC  # 7808


class _Null:
    def __getattr__(self, _name):
        return lambda *a, **k: None


def build_nc(bc=BC, repeat=1, loop_repeat=1, parts=None,
             diff_eng="dve", combine_eng="dve", mb=2, ib=2, ob=2, xf=False):
    """Single-core program for a [bc, T, F] shard.

    parts (timing-only) picks instruction subsets from
    {"dma", "dve", "act", "mz", "gp"}; None = the real kernel."""
    on = {"dma", "dve", "act", "mz", "gp", "scan"} if parts is None else set(parts)
    import concourse.bacc as bacc
    import concourse.mybir as mybir
    from concourse import tile
    from concourse.ap import AP

    f32 = mybir.dt.float32
    bf16 = mybir.dt.bfloat16
    Alu = mybir.AluOpType
    Act = mybir.ActivationFunctionType
    n_tiles = bc // ROWS
    assert n_tiles * ROWS == bc

    nc = bacc.Bacc(
        "TRN2", target_bir_lowering=False, debug=False, num_devices=N_CORES
    )
    pose = nc.dram_tensor("pose", [bc, FT], f32, kind="ExternalInput")
    dts = nc.dram_tensor("dt", [bc, T], f32, kind="ExternalInput")
    outd = nc.dram_tensor("out", [bc, T * OC], f32, kind="ExternalOutput")

    def mk(t, off, pairs):
        return AP(t.tensor, t.offset + off, [list(t.ap[0])] + [list(p) for p in pairs])

    with tile.TileContext(nc) as tc:
        with (
            tc.tile_pool(name="cpool", bufs=1) as cpool,
            tc.tile_pool(name="pin", bufs=ib) as pin,
            tc.tile_pool(name="pmid", bufs=mb) as pmid,
            tc.tile_pool(name="pout", bufs=ob) as pout,
            tc.tile_pool(name="psmall", bufs=2) as psmall,
        ):
            # scan decay tile: Q everywhere, 0 at segment starts (pos % 64
            # == 0).  The same forward-ordered d0 AP serves both scan
            # directions: at iteration j, d0 = qt[j], zero exactly at each
            # segment's first step.
            qt = cpool.tile([128, W], bf16)
            nc.vector.memset(qt[:, :], Q)
            nc.vector.memset(mk(qt, 0, [[FT, CPT], [T, F]]), 0.0)

            # rr_all = bf16(0.5*alpha / clip(dt)) for the whole shard, once
            n_t = bc // ROWS
            dt_all = cpool.tile([128, n_t * CPT * T], f32)
            rrf_all = cpool.tile([128, n_t * CPT * T], f32)
            rr_all = cpool.tile([128, n_t * CPT * T], bf16)
            if parts is None or "dma" in on:
                nc.scalar.dma_start(
                    out=dt_all[:, :],
                    in_=dts[:, :].rearrange("(i c p) t -> p i c t", p=128, c=CPT),
                )
            else:
                nc.vector.memset(mk(dt_all, 0, [[1, 1]]), 0.01)
            if parts is None or "dve" in on:
                nc.vector.tensor_scalar(
                    out=rrf_all[:, :], in0=dt_all[:, :], scalar1=MAX_DT,
                    scalar2=1e-6, op0=Alu.min, op1=Alu.max,
                )
                nc.vector.reciprocal_approx_fast(out=rrf_all[:, :], in_=rrf_all[:, :])
                nc.vector.tensor_scalar_mul(out=rr_all[:, :], in0=rrf_all[:, :],
                                            scalar1=0.5 * ALPHA)
            else:
                nc.vector.memset(mk(rr_all, 0, [[1, 1]]), 0.0)

            def body():
                n = n_tiles * repeat
                _issue_loads(0)
                for i in range(n):
                    if i + 1 < n:
                        _issue_loads(i + 1)
                    _tile_body(i)

            xin_tiles = {}

            def _issue_loads(i):
                r0 = (i % n_tiles) * ROWS
                xin = pin.tile([128, W], f32, name="xin")
                xin_tiles[i] = xin
                if "dma" in on:
                    pose_sl = pose[r0 : r0 + ROWS, :].rearrange(
                        "(c p) f -> p c f", p=128
                    )
                    nc.scalar.dma_start(
                        out=mk(xin, 0, [[FT, CPT], [1, FT]]), in_=pose_sl
                    )
                else:
                    nc.vector.memset(mk(xin, 0, [[1, 1]]), 0.0)

            def _tile_body(i):
                r0 = (i % n_tiles) * ROWS
                xin = xin_tiles.pop(i)
                xF = pmid.tile([128, W], bf16, name="xF") if xf else None
                av = pmid.tile([128, W], bf16)
                yf = pmid.tile([128, W], bf16)
                yb = pmid.tile([128, W], bf16)
                outt = pout.tile([128, OW], f32)

                if parts is not None:
                    if "gp" not in on and diff_eng == "gp":
                        nc.vector.memset(mk(av, 0, [[1, 1]]), 0.0)
                    if "scan" not in on:
                        nc.vector.memset(mk(yf, 0, [[1, 1]]), 0.0)
                        nc.vector.memset(mk(yb, 0, [[1, 1]]), 0.0)
                    if "dma" in on and not ({"dve", "act", "mz", "gp"} & on):
                        nc.vector.memset(mk(outt, 0, [[1, 1]]), 0.0)


                dve = nc.vector if "dve" in on else _Null()

                # ---- av[c,f,t] = x[c,t,f] - x[c,t-1,f] ----
                if xf:
                    # ACT does the t->f transpose + f32->bf16 cast; DVE diff
                    # is then one flat stride-1 TT.  Elements at segment
                    # starts (j % 64 == 0) get cross-segment garbage, which
                    # the t=0 memset below overwrites.
                    if "act" in on:
                        nc.scalar.copy(
                            out=mk(xF, 0, [[FT, CPT], [T, F], [1, T]]),
                            in_=mk(xin, 0, [[FT, CPT], [1, F], [F, T]]),
                        )
                    elif parts is not None:
                        nc.vector.memset(mk(xF, 0, [[1, 1]]), 0.0)
                    dve.tensor_tensor(
                        out=mk(av, 1, [[1, W - 1]]),
                        in0=mk(xF, 1, [[1, W - 1]]),
                        in1=mk(xF, 0, [[1, W - 1]]),
                        op=Alu.subtract,
                    )
                else:
                    diff_args = dict(
                        out=mk(av, 1, [[FT, CPT], [T, F], [1, T - 1]]),
                        in0=mk(xin, F, [[FT, CPT], [1, F], [F, T - 1]]),
                        in1=mk(xin, 0, [[FT, CPT], [1, F], [F, T - 1]]),
                        op=Alu.subtract,
                    )
                    if diff_eng == "gp":
                        if "gp" in on:
                            nc.gpsimd.tensor_tensor(**diff_args)
                    else:
                        dve.tensor_tensor(**diff_args)
                dve.memset(mk(av, 0, [[FT, CPT], [T, F]]), 0.0)

                # scale by rr, broadcast over f — full t range so every AP
                # starts 4B-aligned (t=0 is zero, rr finite: 0*rr=0) => 2x
                dve.tensor_tensor(
                    out=mk(av, 0, [[FT, CPT], [T, F], [1, T]]),
                    in0=mk(av, 0, [[FT, CPT], [T, F], [1, T]]),
                    in1=mk(rr_all, (i % n_tiles) * CPT * T,
                           [[T, CPT], [0, F], [1, T]]),
                    op=Alu.mult,
                )

                # ---- forward EMA scan (fp32 state) ----
                sc = nc.vector if "scan" in on else _Null()
                sc.tensor_tensor_scan(
                    out=yf[:, :], data0=qt[:, :], data1=av[:, :], initial=0.0,
                    op0=Alu.mult, op1=Alu.add,
                )
                # backward initial condition: z_{T-1} = 0.5*v_{T-1}, not
                # 0.5*alpha*v — rescale the t=63 element of each segment
                sc.tensor_scalar_mul(
                    out=mk(av, T - 1, [[FT, CPT], [T, F]]),
                    in0=mk(av, T - 1, [[FT, CPT], [T, F]]),
                    scalar1=1.0 / ALPHA,
                )
                # ---- backward scan: negative-stride data1/out, forward d0
                # (reset pattern is position-based, direction-invariant);
                # writing backwards lands yb t-ascending in memory.
                sc.tensor_tensor_scan(
                    out=mk(yb, W - 1, [[-1, W]]),
                    data0=mk(qt, 0, [[1, W]]),
                    data1=mk(av, W - 1, [[-1, W]]),
                    initial=0.0, op0=Alu.mult, op1=Alu.add,
                )

                # ---- combine: yv = yf + yb (stride-1 bf16 => 2x mode),
                # reusing av's buffer (dead after the bwd scan); the
                # t-major transpose back rides the ACT cast-copy below ----
                yv = av
                dve.tensor_tensor(
                    out=yv[:, :], in0=yf[:, :], in1=yb[:, :], op=Alu.add,
                )
                if "act" in on:
                    nc.scalar.copy(
                        out=mk(outt, F, [[T * OC, CPT], [OC, T], [1, F]]),
                        in_=mk(yv, 0, [[FT, CPT], [1, T], [T, F]]),
                    )

                # ---- spatial passthrough on ACT ----
                if "act" in on:
                    nc.scalar.copy(
                        out=mk(outt, 0, [[T * OC, CPT], [OC, T], [1, F]]),
                        in_=mk(xin, 0, [[FT, CPT], [F, T], [1, F]]),
                    )

                # ---- M-Zeni channel ----
                if "mz" in on:
                    pd = psmall.tile([128, CPT * 3], f32)
                    sqd = psmall.tile([128, CPT * 3], f32)
                    nsq = psmall.tile([128, CPT], f32)
                    inv = psmall.tile([128, CPT], f32)
                    ad = psmall.tile([128, CPT * T * 3], f32)
                    u = psmall.tile([128, CPT * T], f32)
                    mzt = psmall.tile([128, CPT * T], f32)

                    pd3 = mk(pd, 0, [[3, CPT], [1, 3]])
                    nc.vector.tensor_tensor(
                        out=pd3,
                        in0=mk(xin, (T - 1) * F, [[FT, CPT], [1, 3]]),
                        in1=mk(xin, 0, [[FT, CPT], [1, 3]]),
                        op=Alu.subtract,
                    )
                    # nsq[c] = sum_d pd^2 (ACT square + accumulate, per c)
                    for c in range(CPT):
                        nc.scalar.activation(
                            out=mk(sqd, c * 3, [[1, 3]]),
                            in_=mk(pd, c * 3, [[1, 3]]),
                            func=Act.Square,
                            accum_out=mk(nsq, c, [[1, 1]]),
                        )
                    # inv = 1 / (sqrt(nsq) + 63e-6)
                    nc.scalar.sqrt(out=mk(nsq, 0, [[1, CPT]]),
                                   in_=mk(nsq, 0, [[1, CPT]]))
                    nc.vector.tensor_scalar_add(
                        out=mk(nsq, 0, [[1, CPT]]), in0=mk(nsq, 0, [[1, CPT]]),
                        scalar1=(T - 1) * 1e-6,
                    )
                    nc.vector.reciprocal_approx_fast(
                        out=mk(inv, 0, [[1, CPT]]), in_=mk(nsq, 0, [[1, CPT]]))
                    # u[c,t] = sum_d (al - ar) * pd   (unnormalized dot)
                    ad3 = mk(ad, 0, [[T * 3, CPT], [3, T], [1, 3]])
                    nc.vector.tensor_tensor(
                        out=ad3,
                        in0=mk(xin, 9, [[FT, CPT], [F, T], [1, 3]]),
                        in1=mk(xin, 18, [[FT, CPT], [F, T], [1, 3]]),
                        op=Alu.subtract,
                    )
                    nc.vector.tensor_tensor(
                        out=ad3, in0=ad3,
                        in1=mk(pd, 0, [[3, CPT], [0, T], [1, 3]]),
                        op=Alu.mult,
                    )
                    nc.vector.tensor_reduce(
                        out=mk(u, 0, [[T, CPT], [1, T]]),
                        in_=ad3, axis=mybir.AxisListType.X, op=Alu.add,
                    )
                    # mz = u * inv + 1e-6  -> channel 60
                    nc.vector.tensor_tensor(
                        out=mk(mzt, 0, [[T, CPT], [1, T]]),
                        in0=mk(u, 0, [[T, CPT], [1, T]]),
                        in1=mk(inv, 0, [[1, CPT], [0, T]]),
                        op=Alu.mult,
                    )
                    nc.vector.tensor_scalar_add(
                        out=mk(outt, 2 * F, [[T * OC, CPT], [OC, T]]),
                        in0=mk(mzt, 0, [[T, CPT], [1, T]]), scalar1=1e-6,
                    )

                # ---- store (SP-issued HWDGE ring) ----
                if "dma" in on:
                    out_sl = outd[r0 : r0 + ROWS, :].rearrange(
                        "(c p) f -> p c f", p=128
                    )
                    nc.sync.dma_start(
                        out=out_sl, in_=mk(outt, 0, [[T * OC, CPT], [1, T * OC]])
                    )

            if loop_repeat > 1:
                with tc.For_i(0, loop_repeat, 1):
                    body()
            else:
                body()

    nc.compile()
    return nc


_CACHE = {}


def _get_nc():
    if "nc" not in _CACHE:
        _CACHE["nc"] = build_nc(BC)
    return _CACHE["nc"]


def kernel(pose_seq: np.ndarray, dt_seq: np.ndarray) -> np.ndarray:
    from concourse.bass_utils import run_bass_kernel_spmd

    nc = _get_nc()
    pose = np.ascontiguousarray(pose_seq.reshape(B, FT), dtype=np.float32)
    dt = np.ascontiguousarray(dt_seq.reshape(B, T), dtype=np.float32)

    in_maps = [
        {
            "pose": pose[c * BC : (c + 1) * BC],
            "dt": dt[c * BC : (c + 1) * BC],
        }
        for c in range(N_CORES)
    ]
    res = run_bass_kernel_spmd(nc, in_maps, list(range(N_CORES)))
    out = np.concatenate([r["out"] for r in res.results], axis=0)
    return out.reshape(B, T, OC)


# revision 3
# speedup vs baseline: 1.0120x; 1.0021x over previous
"""Trainium2 Bass kernel for nn_PhysicsPriorExtractor — bf16 rework.

Per batch element (B=32768, T=64, K*D=30 features):
  spatial = pose reshaped [T, 30]                          -> out channels 0..29
  vel     = bidirectional-EMA(diff(spatial)/clip(dt))      -> out channels 30..59
  mzeni   = dot(ankle_l - ankle_r, forward_dir) + 1e-6     -> out channel 60

Measured op rates (this box): DVE scan 2.37 cyc/elem any dtype; DVE TT
bf16 stride-1 2x (0.53 cyc/elem); strided TT ~2 cyc/elem; GPSIMD TT
7.2-13.2 us per 3840-elem op, ~40-60% hideable behind DVE.

Engine split per tile (CPT=2 batch rows/partition, 16 tiles/core):
  DVE : rr chain (clip/recip_approx/cast), the 2x stride-1 scale,
        fwd scan, t=63 patch, bwd scan (negative-stride APs — no reverse
        copy, output lands t-ascending), combine(+transpose) into outt,
        small mz ops
  GP  : diff (fused t->f transpose + f32->bf16 cast)
  ACT : spatial passthrough copy, mz sqrt, mz square+accum
  DMA : HWDGE loads (ACT ring), HWDGE store (SP ring)
"""

import sys

if "/opt/trn_rl_repo" not in sys.path:
    sys.path.insert(0, "/opt/trn_rl_repo")

import numpy as np

B, T, F = 32768, 64, 30
OC = 2 * F + 1
N_CORES = 8
BC = B // N_CORES
CPT = 2
ROWS = 128 * CPT
ALPHA = 0.7
Q = 1.0 - ALPHA
MAX_DT = 0.1

FT = F * T  # 1920
W = CPT * FT  # 3840
OW = CPT * T * OC  # 7808


class _Null:
    def __getattr__(self, _name):
        return lambda *a, **k: None


def build_nc(bc=BC, repeat=1, loop_repeat=1, parts=None,
             diff_eng="dve", combine_eng="dve", mb=2, ib=2, ob=2, xf=False):
    """Single-core program for a [bc, T, F] shard.

    parts (timing-only) picks instruction subsets from
    {"dma", "dve", "act", "mz", "gp"}; None = the real kernel."""
    on = {"dma", "dve", "act", "mz", "gp", "scan"} if parts is None else set(parts)
    import concourse.bacc as bacc
    import concourse.mybir as mybir
    from concourse import tile
    from concourse.ap import AP

    f32 = mybir.dt.float32
    bf16 = mybir.dt.bfloat16
    Alu = mybir.AluOpType
    Act = mybir.ActivationFunctionType
    n_tiles = bc // ROWS
    assert n_tiles * ROWS == bc

    nc = bacc.Bacc(
        "TRN2", target_bir_lowering=False, debug=False, num_devices=N_CORES
    )
    pose = nc.dram_tensor("pose", [bc, FT], f32, kind="ExternalInput")
    dts = nc.dram_tensor("dt", [bc, T], f32, kind="ExternalInput")
    outd = nc.dram_tensor("out", [bc, T * OC], f32, kind="ExternalOutput")

    def mk(t, off, pairs):
        return AP(t.tensor, t.offset + off, [list(t.ap[0])] + [list(p) for p in pairs])

    with tile.TileContext(nc) as tc:
        with (
            tc.tile_pool(name="cpool", bufs=1) as cpool,
            tc.tile_pool(name="pin", bufs=ib) as pin,
            tc.tile_pool(name="pmid", bufs=mb) as pmid,
            tc.tile_pool(name="pout", bufs=ob) as pout,
            tc.tile_pool(name="psmall", bufs=2) as psmall,
        ):
            # scan decay tile: Q everywhere, 0 at segment starts (pos % 64
            # == 0).  The same forward-ordered d0 AP serves both scan
            # directions: at iteration j, d0 = qt[j], zero exactly at each
            # segment's first step.
            qt = cpool.tile([128, W], bf16)
            nc.vector.memset(qt[:, :], Q)
            nc.vector.memset(mk(qt, 0, [[FT, CPT], [T, F]]), 0.0)

            # rr_all = bf16(0.5*alpha / clip(dt)) for the whole shard, once
            n_t = bc // ROWS
            dt_all = cpool.tile([128, n_t * CPT * T], f32)
            rrf_all = cpool.tile([128, n_t * CPT * T], f32)
            rr_all = cpool.tile([128, n_t * CPT * T], bf16)
            if parts is None or "dma" in on:
                nc.scalar.dma_start(
                    out=dt_all[:, :],
                    in_=dts[:, :].rearrange("(i c p) t -> p i c t", p=128, c=CPT),
                )
            else:
                nc.vector.memset(mk(dt_all, 0, [[1, 1]]), 0.01)
            if parts is None or "dve" in on:
                nc.vector.tensor_scalar(
                    out=rrf_all[:, :], in0=dt_all[:, :], scalar1=MAX_DT,
                    scalar2=1e-6, op0=Alu.min, op1=Alu.max,
                )
                nc.vector.reciprocal_approx_fast(out=rrf_all[:, :], in_=rrf_all[:, :])
                nc.vector.tensor_scalar_mul(out=rr_all[:, :], in0=rrf_all[:, :],
                                            scalar1=0.5 * ALPHA)
            else:
                nc.vector.memset(mk(rr_all, 0, [[1, 1]]), 0.0)

            def body():
                n = n_tiles * repeat
                _issue_loads(0)
                for i in range(n):
                    if i + 1 < n:
                        _issue_loads(i + 1)
                    _tile_body(i)

            xin_tiles = {}

            def _issue_loads(i):
                r0 = (i % n_tiles) * ROWS
                xin = pin.tile([128, W], f32, name="xin")
                xin_tiles[i] = xin
                if "dma" in on:
                    pose_sl = pose[r0 : r0 + ROWS, :].rearrange(
                        "(c p) f -> p c f", p=128
                    )
                    nc.scalar.dma_start(
                        out=mk(xin, 0, [[FT, CPT], [1, FT]]), in_=pose_sl
                    )
                else:
                    nc.vector.memset(mk(xin, 0, [[1, 1]]), 0.0)

            def _tile_body(i):
                r0 = (i % n_tiles) * ROWS
                xin = xin_tiles.pop(i)
                xF = pmid.tile([128, W], bf16, name="xF") if xf else None
                av = pmid.tile([128, W], bf16)
                yf = pmid.tile([128, W], bf16)
                yb = pmid.tile([128, W], bf16)
                outt = pout.tile([128, OW], f32)

                if parts is not None:
                    if "gp" not in on and diff_eng == "gp":
                        nc.vector.memset(mk(av, 0, [[1, 1]]), 0.0)
                    if "scan" not in on:
                        nc.vector.memset(mk(yf, 0, [[1, 1]]), 0.0)
                        nc.vector.memset(mk(yb, 0, [[1, 1]]), 0.0)
                    if "dma" in on and not ({"dve", "act", "mz", "gp"} & on):
                        nc.vector.memset(mk(outt, 0, [[1, 1]]), 0.0)


                dve = nc.vector if "dve" in on else _Null()

                # ---- av[c,f,t] = x[c,t,f] - x[c,t-1,f] ----
                if xf:
                    # ACT does the t->f transpose + f32->bf16 cast; DVE diff
                    # is then one flat stride-1 TT.  Elements at segment
                    # starts (j % 64 == 0) get cross-segment garbage, which
                    # the t=0 memset below overwrites.
                    if "act" in on:
                        nc.scalar.copy(
                            out=mk(xF, 0, [[FT, CPT], [T, F], [1, T]]),
                            in_=mk(xin, 0, [[FT, CPT], [1, F], [F, T]]),
                        )
                    elif parts is not None:
                        nc.vector.memset(mk(xF, 0, [[1, 1]]), 0.0)
                    dve.tensor_tensor(
                        out=mk(av, 1, [[1, W - 1]]),
                        in0=mk(xF, 1, [[1, W - 1]]),
                        in1=mk(xF, 0, [[1, W - 1]]),
                        op=Alu.subtract,
                    )
                else:
                    diff_args = dict(
                        out=mk(av, 1, [[FT, CPT], [T, F], [1, T - 1]]),
                        in0=mk(xin, F, [[FT, CPT], [1, F], [F, T - 1]]),
                        in1=mk(xin, 0, [[FT, CPT], [1, F], [F, T - 1]]),
                        op=Alu.subtract,
                    )
                    if diff_eng == "gp":
                        if "gp" in on:
                            nc.gpsimd.tensor_tensor(**diff_args)
                    else:
                        dve.tensor_tensor(**diff_args)
                dve.memset(mk(av, 0, [[FT, CPT], [T, F]]), 0.0)

                # scale by rr, broadcast over f — full t range so every AP
                # starts 4B-aligned (t=0 is zero, rr finite: 0*rr=0) => 2x
                dve.tensor_tensor(
                    out=mk(av, 0, [[FT, CPT], [T, F], [1, T]]),
                    in0=mk(av, 0, [[FT, CPT], [T, F], [1, T]]),
                    in1=mk(rr_all, (i % n_tiles) * CPT * T,
                           [[T, CPT], [0, F], [1, T]]),
                    op=Alu.mult,
                )

                # ---- forward EMA scan (fp32 state) ----
                sc = nc.vector if "scan" in on else _Null()
                sc.tensor_tensor_scan(
                    out=yf[:, :], data0=qt[:, :], data1=av[:, :], initial=0.0,
                    op0=Alu.mult, op1=Alu.add,
                )
                # backward initial condition: z_{T-1} = 0.5*v_{T-1}, not
                # 0.5*alpha*v — rescale the t=63 element of each segment
                sc.tensor_scalar_mul(
                    out=mk(av, T - 1, [[FT, CPT], [T, F]]),
                    in0=mk(av, T - 1, [[FT, CPT], [T, F]]),
                    scalar1=1.0 / ALPHA,
                )
                # ---- backward scan: negative-stride data1/out, forward d0
                # (reset pattern is position-based, direction-invariant);
                # writing backwards lands yb t-ascending in memory.
                sc.tensor_tensor_scan(
                    out=mk(yb, W - 1, [[-1, W]]),
                    data0=mk(qt, 0, [[1, W]]),
                    data1=mk(av, W - 1, [[-1, W]]),
                    initial=0.0, op0=Alu.mult, op1=Alu.add,
                )

                # ---- combine: yv = yf + yb (stride-1 bf16 => 2x mode),
                # reusing av's buffer (dead after the bwd scan); the
                # t-major transpose back rides the ACT cast-copy below ----
                yv = av
                dve.tensor_tensor(
                    out=yv[:, :], in0=yf[:, :], in1=yb[:, :], op=Alu.add,
                )
                if "act" in on:
                    nc.scalar.copy(
                        out=mk(outt, F, [[T * OC, CPT], [OC, T], [1, F]]),
                        in_=mk(yv, 0, [[FT, CPT], [1, T], [T, F]]),
                    )

                # ---- spatial passthrough on ACT ----
                if "act" in on:
                    nc.scalar.copy(
                        out=mk(outt, 0, [[T * OC, CPT], [OC, T], [1, F]]),
                        in_=mk(xin, 0, [[FT, CPT], [F, T], [1, F]]),
                    )

                # ---- M-Zeni channel ----
                if "mz" in on:
                    pd = psmall.tile([128, CPT * 3], f32)
                    sqd = psmall.tile([128, CPT * 3], f32)
                    nsq = psmall.tile([128, CPT], f32)
                    inv = psmall.tile([128, CPT], f32)
                    ad = psmall.tile([128, CPT * T * 3], f32)
                    u = psmall.tile([128, CPT * T], f32)

                    pd3 = mk(pd, 0, [[3, CPT], [1, 3]])
                    nc.vector.tensor_tensor(
                        out=pd3,
                        in0=mk(xin, (T - 1) * F, [[FT, CPT], [1, 3]]),
                        in1=mk(xin, 0, [[FT, CPT], [1, 3]]),
                        op=Alu.subtract,
                    )
                    # nsq[c] = sum_d pd^2 (ACT square + accumulate, per c)
                    for c in range(CPT):
                        nc.scalar.activation(
                            out=mk(sqd, c * 3, [[1, 3]]),
                            in_=mk(pd, c * 3, [[1, 3]]),
                            func=Act.Square,
                            accum_out=mk(nsq, c, [[1, 1]]),
                        )
                    # inv = 1 / (sqrt(nsq) + 63e-6)
                    nc.scalar.sqrt(out=mk(nsq, 0, [[1, CPT]]),
                                   in_=mk(nsq, 0, [[1, CPT]]))
                    nc.vector.tensor_scalar_add(
                        out=mk(nsq, 0, [[1, CPT]]), in0=mk(nsq, 0, [[1, CPT]]),
                        scalar1=(T - 1) * 1e-6,
                    )
                    nc.vector.reciprocal_approx_fast(
                        out=mk(inv, 0, [[1, CPT]]), in_=mk(nsq, 0, [[1, CPT]]))
                    # u[c,t] = sum_d (al - ar) * pd   (unnormalized dot)
                    ad3 = mk(ad, 0, [[T * 3, CPT], [3, T], [1, 3]])
                    nc.vector.tensor_tensor(
                        out=ad3,
                        in0=mk(xin, 9, [[FT, CPT], [F, T], [1, 3]]),
                        in1=mk(xin, 18, [[FT, CPT], [F, T], [1, 3]]),
                        op=Alu.subtract,
                    )
                    nc.vector.tensor_tensor(
                        out=ad3, in0=ad3,
                        in1=mk(pd, 0, [[3, CPT], [0, T], [1, 3]]),
                        op=Alu.mult,
                    )
                    nc.vector.tensor_reduce(
                        out=mk(u, 0, [[T, CPT], [1, T]]),
                        in_=ad3, axis=mybir.AxisListType.X, op=Alu.add,
                    )
                    # mz = u * inv + 1e-6 -> channel 60, on ACT:
                    # activation computes Copy(in*scale + bias) with a
                    # per-partition scalar AP as scale (one op per c)
                    for c in range(CPT):
                        nc.scalar.activation(
                            out=mk(outt, 2 * F + c * T * OC, [[OC, T]]),
                            in_=mk(u, c * T, [[1, T]]),
                            func=Act.Copy,
                            scale=mk(inv, c, [[1, 1]]),
                            bias=1e-6,
                        )

                # ---- store (SP-issued HWDGE ring) ----
                if "dma" in on:
                    out_sl = outd[r0 : r0 + ROWS, :].rearrange(
                        "(c p) f -> p c f", p=128
                    )
                    nc.sync.dma_start(
                        out=out_sl, in_=mk(outt, 0, [[T * OC, CPT], [1, T * OC]])
                    )

            if loop_repeat > 1:
                with tc.For_i(0, loop_repeat, 1):
                    body()
            else:
                body()

    nc.compile()
    return nc


_CACHE = {}


def _get_nc():
    if "nc" not in _CACHE:
        _CACHE["nc"] = build_nc(BC)
    return _CACHE["nc"]


def kernel(pose_seq: np.ndarray, dt_seq: np.ndarray) -> np.ndarray:
    from concourse.bass_utils import run_bass_kernel_spmd

    nc = _get_nc()
    pose = np.ascontiguousarray(pose_seq.reshape(B, FT), dtype=np.float32)
    dt = np.ascontiguousarray(dt_seq.reshape(B, T), dtype=np.float32)

    in_maps = [
        {
            "pose": pose[c * BC : (c + 1) * BC],
            "dt": dt[c * BC : (c + 1) * BC],
        }
        for c in range(N_CORES)
    ]
    res = run_bass_kernel_spmd(nc, in_maps, list(range(N_CORES)))
    out = np.concatenate([r["out"] for r in res.results], axis=0)
    return out.reshape(B, T, OC)
